# revision 50
# baseline (speedup 1.0000x reference)
"""Distributed GQA attention kernel for 8 TRN2 NeuronCores.

Sharding: core c = 4*b + k handles batch b (of 2) and kv-head k (of 4),
i.e. Q heads 4k..4k+3 (column-parallel qkv).  Attention is computed per
core in transposed layout (S^T = K Q^T per 128-key tile, causal-skipped).
The output projection is ROW-parallel: each core contracts only its own
four heads' attention output against its 512 rows of wo, producing a
full [T, 2048] partial (bf16); the host sums the four partials of each
batch group.  No device collectives at all.

All matmuls run in bf16 (fp32 PSUM accumulation).  Softmax row sums are
accumulated as a bf16 partial-sum tile on the DVE; the partition
reduction + broadcast runs on the otherwise-idle GPSIMD engine
(partition_all_reduce, attn ucode library), so the PE never serializes
against the softmax tail.  Next-chunk projection chains and
previous-chunk o_proj groups are interleaved into the attention head
loop as PE filler so the TensorEngine never idles on the softmax
(ACT/DVE) pipeline.  o_proj groups keep the stationary operand (an
attention-output 128-column slice) across pairs of matmuls so weight
loads amortize.
"""
import sys
import numpy as np

for _p in ("/root/.axon_site", "/root/.axon_site/_ro/trn_rl_repo",
           "/root/.axon_site/_ro/pypackages"):
    if _p not in sys.path:
        sys.path.append(_p)

import ml_dtypes  # noqa: E402
import concourse.bass as bass  # noqa: E402
from concourse import bacc  # noqa: E402
import concourse.mybir as mybir  # noqa: E402
import concourse.bass_isa as bass_isa  # noqa: E402
from concourse import tile  # noqa: E402
from concourse import library_config  # noqa: E402
import concourse.bass_utils as bass_utils  # noqa: E402

F32 = mybir.dt.float32
BF16 = mybir.dt.bfloat16
AL = mybir.AluOpType
ACTF = mybir.ActivationFunctionType
BF16NP = ml_dtypes.bfloat16

B, T, D = 2, 2048, 2048
H, HK, HD = 16, 4, 128
HPC = 4                      # q-heads per core
QCOLS = HPC * HD             # 512 q columns per core
CHUNK = 512                  # t-chunk
NE = D // 128                # contraction e-chunks
THETA = 10000.0
SCALE = 1.0 / float(np.sqrt(HD))
N_CORES = 8


def _consts(t=T):
    freqs = 1.0 / THETA ** (np.arange(0, HD, 2, dtype=np.float64) / HD)
    pos = np.arange(t, dtype=np.float64)
    ang = np.outer(freqs, pos)                            # [64, t]
    cos = np.cos(ang).astype(np.float32)
    sin = np.sin(ang).astype(np.float32)
    cos_full = np.concatenate([cos, cos], axis=0).astype(BF16NP)  # [128, t]
    sin_pm = np.concatenate([-sin, sin], axis=0).astype(BF16NP)   # [128, t]
    swap = np.zeros((128, 128), np.float32)
    swap[(np.arange(128) + 64) % 128, np.arange(128)] = 1.0
    ident = np.eye(128, dtype=np.float32)
    # triangular causal mask for the 128-col strip where a diagonal key
    # tile crosses the query range: tri[pp, jj] = 1 iff pp <= jj
    tri = (np.arange(128)[:, None] <= np.arange(128)[None, :]).astype(
        np.float32)
    return (cos_full, sin_pm, swap.astype(BF16NP), ident.astype(BF16NP),
            tri.astype(BF16NP))


def build(t=T):
    nchunk = t // CHUNK
    npt = t // 128
    nc = bacc.Bacc("TRN2", target_bir_lowering=False, debug=False,
                   num_devices=N_CORES)
    # inputs arrive e-major-tiled ([128, NE*cols]: row p, block e holds
    # original row 128e+p) so every DMA moves >=2KB-contiguous runs: the
    # input phase is DMA packet-rate-bound, not byte-bound
    xT_e = nc.declare_dram_parameter("xT", [128, NE * t], BF16,
                                     isOutput=False)
    wq_e = nc.declare_dram_parameter("wq", [128, NE * QCOLS], BF16,
                                     isOutput=False)
    wk_e = nc.declare_dram_parameter("wk", [128, NE * HD], BF16,
                                     isOutput=False)
    wv_e = nc.declare_dram_parameter("wv", [128, NE * HD], BF16,
                                     isOutput=False)
    wo_e = nc.declare_dram_parameter("wo", [QCOLS, D], BF16, isOutput=False)
    out_e = nc.declare_dram_parameter("out", [t, D], BF16, isOutput=True)

    cos_np, sinpm_np, swap_np, ident_np, tri_np = _consts(t)
    cos_d = nc.inline_tensor(cos_np, "cos_c")
    sinpm_d = nc.inline_tensor(sinpm_np, "sinpm_c")
    swap_d = nc.inline_tensor(swap_np, "swap_c")
    ident_d = nc.inline_tensor(ident_np, "ident_c")
    tri_d = nc.inline_tensor(tri_np, "tri_c")

    with tile.TileContext(nc) as tc:
        with (
            tc.tile_pool(name="wpool", bufs=1) as wpool,
            tc.tile_pool(name="xpool", bufs=32) as xpool,
            tc.tile_pool(name="kvpool", bufs=1) as kvpool,
            tc.tile_pool(name="work", bufs=2) as work,
            tc.tile_pool(name="ptpool", bufs=8) as ptpool,
            tc.tile_pool(name="aopool", bufs=8) as aopool,
            tc.tile_pool(name="osbpool", bufs=2) as osbpool,
            tc.tile_pool(name="qkv_ps", bufs=2, space="PSUM") as qkv_ps,
            tc.tile_pool(name="sc_ps", bufs=2, space="PSUM") as sc_ps,
            tc.tile_pool(name="av_ps", bufs=1, space="PSUM") as av_ps,
            tc.tile_pool(name="sm_ps", bufs=1, space="PSUM") as sm_ps,
            tc.tile_pool(name="op_ps", bufs=2, space="PSUM") as op_ps,
        ):
            nc.gpsimd.load_library(library_config.attn)
            # ---- DMA plan: weights in e-group pieces (the first unlocks
            # the prologue after ~1us), full-T x tiles per e, then consts
            # and (lazily) wo.  Everything is >=1KB-contiguous per row.
            nq = max(1, NE // 4)
            pw = min(2 * CHUNK, t)          # chunk-pair width
            npair = t // pw
            wk_t = wpool.tile([128, NE * HD], BF16, name="wk_t")
            wv_t = wpool.tile([128, NE * HD], BF16, name="wv_t")
            wq_t = wpool.tile([128, NE * QCOLS], BF16, name="wq_t")
            xp = [[xpool.tile([128, pw], BF16, name=f"xe{P}_{e}", tag="xt")
                   for e in range(NE)] for P in range(npair)]
            half = NE * HD // 2
            nc.sync.dma_start(wk_t[:, 0:half], wk_e[:, 0:half])
            nc.sync.dma_start(wv_t[:, 0:half], wv_e[:, 0:half])
            for j in range(nq):
                qw = 4 * QCOLS
                nc.sync.dma_start(wq_t[:, qw * j:qw * (j + 1)],
                                  wq_e[:, qw * j:qw * (j + 1)])
                for e in range(4 * j, min(4 * (j + 1), NE)):
                    nc.sync.dma_start(xp[0][e][:],
                                      xT_e[:, pw * e:pw * (e + 1)])
                if j == 0:
                    nc.sync.dma_start(wk_t[:, half:], wk_e[:, half:])
                    nc.sync.dma_start(wv_t[:, half:], wv_e[:, half:])

            swap_sb = wpool.tile([128, 128], BF16, name="swap_sb")
            nc.sync.dma_start(swap_sb[:], swap_d[:, :])
            ident_sb = wpool.tile([128, 128], BF16, name="ident_sb")
            nc.sync.dma_start(ident_sb[:], ident_d[:, :])
            tri_sb = wpool.tile([128, 128], BF16, name="tri_sb")
            nc.sync.dma_start(tri_sb[:], tri_d[:, :])
            cos_sb = wpool.tile([128, t], BF16, name="cos_sb")
            nc.sync.dma_start(cos_sb[:], cos_d[:, :])
            sinpm_sb = wpool.tile([128, t], BF16, name="sinpm_sb")
            nc.sync.dma_start(sinpm_sb[:], sinpm_d[:, :])
            for P in range(1, npair):
                for e in range(NE):
                    base = NE * pw * P + pw * e
                    nc.sync.dma_start(xp[P][e][:],
                                      xT_e[:, base:base + pw])

            cpp = pw // CHUNK               # chunks per pair
            xts_by_chunk = {
                c: [xp[c // cpp][e][:, (c % cpp) * CHUNK:
                                    (c % cpp + 1) * CHUNK]
                    for e in range(NE)]
                for c in range(nchunk)}
            wo_sb = []

            kT_sb = kvpool.tile([128, t], BF16, name="kT_sb")
            v_tiles = [kvpool.tile([128, HD], BF16, name=f"v{i}")
                       for i in range(npt)]

            def rope_fin(qsb, out_ap, cols):
                """half-swap matmul + the two rotation multiplies"""
                qsw = sm_ps.tile([128, CHUNK], F32, tag="sm")
                nc.tensor.matmul(qsw[:], swap_sb[:], qsb[:],
                                 start=True, stop=True)
                t1 = work.tile([128, CHUNK], BF16, tag="ropeb")
                nc.vector.tensor_tensor(t1[:], qsb[:], cos_sb[:, cols], AL.mult)
                t2 = work.tile([128, CHUNK], BF16, tag="ropec")
                nc.vector.tensor_tensor(t2[:], qsw[:], sinpm_sb[:, cols],
                                        AL.mult)
                nc.vector.tensor_tensor(out_ap, t1[:], t2[:], AL.add)

            def load_wo():
                if not wo_sb:
                    for h in range(HPC):
                        wo_t = wpool.tile([128, D], BF16, name=f"wo{h}")
                        nc.sync.dma_start(wo_t[:],
                                          wo_e[128 * h:128 * (h + 1), :])
                        wo_sb.append(wo_t)

            osb_cur = {}
            in_head = [False]

            def emit_oproj_group(tcx, aos, j, q):
                """one [128 t-rows x 512 cols] block of the o_proj partial:
                accumulate the 4 heads; quarters aggregate into one
                [128, 2048] row-block so the store DMA moves 4KB rows."""
                while pending_tails and pending_tails[0][1] <= tcx:
                    drain_tail(force=True)
                ps = op_ps.tile([128, CHUNK], F32, tag="op")
                for h in range(HPC):
                    nc.tensor.matmul(ps[:], aos[h][:, 128 * j:128 * (j + 1)],
                                     wo_sb[h][:, CHUNK * q:CHUNK * (q + 1)],
                                     start=(h == 0), stop=(h == HPC - 1))
                if q == 0:
                    osb_cur[(tcx, j)] = osbpool.tile(
                        [128, D], BF16, tag="osb", name=f"osb{tcx}_{j}")
                osb = osb_cur[(tcx, j)]
                # drain on DVE while attention heads run (the exp stream
                # makes the ACT queue multi-us deep there); alternate
                # ACT/DVE in the inter-head bulk
                if in_head[0] or (j + q) % 2 == 1:
                    nc.vector.tensor_copy(osb[:, CHUNK * q:CHUNK * (q + 1)],
                                          ps[:])
                else:
                    nc.scalar.copy(osb[:, CHUNK * q:CHUNK * (q + 1)], ps[:])
                if q == 3:
                    row = tcx * CHUNK + 128 * j
                    nc.sync.dma_start(out_e[row:row + 128, :], osb[:])
                    del osb_cur[(tcx, j)]

            def chain_mm(ps, kind, h, xts_c, e):
                if kind == "q":
                    base = QCOLS * e + 128 * h
                    w_ap = wq_t[:, base:base + 128]
                elif kind == "k":
                    w_ap = wk_t[:, HD * e:HD * (e + 1)]
                else:
                    w_ap = wv_t[:, HD * e:HD * (e + 1)]
                nc.tensor.matmul(ps[:], w_ap, xts_c[e],
                                 start=(e == 0), stop=(e == NE - 1))

            def chain_copy(ps, kind):
                """PSUM -> bf16 SBUF drain of a finished chain (frees the
                chain bank; the PE-side finish is deferred)."""
                tag = "vsb" if kind == "v" else "ropea"
                sb = work.tile([128, CHUNK], BF16, tag=tag, bufs=6)
                nc.vector.tensor_copy(sb[:], ps[:])
                return sb

            def chain_fin(cidx, kind, h, sb, qT_list):
                """deferred PE-side finish: rope (q/k) or transposes (v).
                Runs a few filler slots after the chain so its DVE drain has
                cleared the queue and the PE never waits on it."""
                ccols = slice(cidx * CHUNK, (cidx + 1) * CHUNK)
                if kind == "v":
                    for j in range(4):
                        tp = sm_ps.tile([128, 128], BF16, tag="sm")
                        nc.tensor.transpose(
                            tp[:], sb[:, 128 * j:128 * (j + 1)],
                            ident_sb[:])
                        nc.vector.tensor_copy(
                            v_tiles[4 * cidx + j][:], tp[:])
                elif kind == "q":
                    qT = work.tile([128, CHUNK], BF16, tag="qT",
                                   bufs=10, name=f"qT{cidx}_{h}")
                    qT_list[h] = qT
                    rope_fin(sb, qT[:], ccols)
                else:
                    rope_fin(sb, kT_sb[:, ccols], ccols)

            def emit_chain(cidx, kind, h, xts_c, qT_list, defer=None):
                """one projection accumulation chain; the rope/transpose
                finish is appended to `defer` (or emitted inline)."""
                ps = qkv_ps.tile([128, CHUNK], F32, tag="qkv",
                                 name=f"ps_{cidx}_{kind}{h}")
                for e in range(NE):
                    chain_mm(ps, kind, h, xts_c, e)
                sb = chain_copy(ps, kind)
                fin = lambda: chain_fin(cidx, kind, h, sb, qT_list)  # noqa
                if defer is None:
                    fin()
                else:
                    defer.append(fin)

            CHAIN_ORDER = [("k", 0), ("v", 0)] + [("q", h) for h in range(HPC)]
            pending_oproj = []
            pending_tails = []
            ghead = [0]
            qT_next = [None] * HPC

            def tail_a(tcx, h, aos, avp, sumA):
                """partition-reduce + broadcast the softmax row sums on the
                GPSIMD engine; free the AV PSUM bank with an ACT copy."""
                sums_bc = work.tile([128, CHUNK], F32, tag="sumbc",
                                    bufs=4, name=f"sbc{tcx}_{h}")
                nc.gpsimd.partition_all_reduce(
                    sums_bc[:], sumA[:], channels=128,
                    reduce_op=bass_isa.ReduceOp.add)
                av_sb = work.tile([128, CHUNK], BF16, tag="avsb",
                                  bufs=4, name=f"avsb{tcx}_{h}")
                nc.scalar.copy(av_sb[:], avp[:])
                pending_tails.append((ghead[0], tcx, h, aos, av_sb, sums_bc))
                ghead[0] += 1

            def tail_b(_idx, tcx, h, aos, av_sb, sums_bc):
                """reciprocal + normalize into the o_proj input tile"""
                recb = work.tile([128, CHUNK], F32, tag="recb",
                                 name=f"recb{tcx}_{h}")
                nc.vector.reciprocal_approx_fast(recb[:], sums_bc[:])
                ao = aopool.tile([128, CHUNK], BF16, tag="ao",
                                 name=f"ao{tcx}_{h}")
                nc.vector.tensor_tensor(ao[:], av_sb[:], recb[:], AL.mult)
                aos[h] = ao

            def drain_tail(min_age=2, force=False):
                if pending_tails and (
                        force or pending_tails[0][0] <= ghead[0] - min_age):
                    tail_b(*pending_tails.pop(0))
            # prologue: all six chunk-0 projection chains run e-major on
            # borrowed (idle) PSUM banks so each arriving DMA group unlocks
            # six matmuls and the PE paces with DMA arrival.  K/Q0/V finish
            # inline (chunk-0 head 0 needs them); Q1-Q3 finishes become the
            # first fillers of chunk 0.
            pro = [("k", 0, qkv_ps.tile([128, CHUNK], F32, tag="qkv",
                                        name="ps_pro_k")),
                   ("q", 0, qkv_ps.tile([128, CHUNK], F32, tag="qkv",
                                        name="ps_pro_q0")),
                   ("v", 0, sc_ps.tile([128, CHUNK], F32, tag="sc",
                                       name="ps_pro_v")),
                   ("q", 1, sc_ps.tile([128, CHUNK], F32, tag="sc",
                                       name="ps_pro_q1")),
                   ("q", 2, op_ps.tile([128, CHUNK], F32, tag="op",
                                       name="ps_pro_q2")),
                   ("q", 3, op_ps.tile([128, CHUNK], F32, tag="op",
                                       name="ps_pro_q3"))]
            for e in range(NE):
                for kind, h, ps in pro:
                    chain_mm(ps, kind, h, xts_by_chunk[0], e)
            pro_sb = {(kind, h): chain_copy(ps, kind)
                      for kind, h, ps in pro}
            pro_fins = []
            for kind, h in (("k", 0), ("q", 0), ("v", 0)):
                chain_fin(0, kind, h, pro_sb[(kind, h)], qT_next)
            for kind, h in (("q", 1), ("q", 2), ("q", 3)):
                def mk_fin(kk=kind, hh=h, ql=qT_next):
                    return lambda: chain_fin(0, kk, hh, pro_sb[(kk, hh)], ql)
                pro_fins.append(mk_fin())

            for tcx in range(nchunk):
                qT_heads = qT_next
                # filler work: PE instructions with no dependence on this
                # chunk's softmax pipeline, spread between attention heads
                fillers = pro_fins
                pro_fins = []
                if tcx + 1 < nchunk:
                    qT_next = [None] * HPC
                    qn = qT_next

                    def mk_chain(kind, ch, xc=xts_by_chunk[tcx + 1], qq=qn,
                                 ci=tcx + 1):
                        return lambda: emit_chain(ci, kind, ch, xc, qq,
                                                  defer=fillers)

                    fillers += [mk_chain(kind, ch) for kind, ch in CHAIN_ORDER]
                while pending_oproj:
                    optcx, opaos = pending_oproj.pop(0)

                    def mk_grp(j, q, ti=optcx, aa=opaos):
                        return lambda: emit_oproj_group(ti, aa, j, q)

                    fillers += [mk_grp(j, q)
                                for j in range(4) for q in range(4)]

                # ---- attention for q-chunk tcx ----
                load_wo()
                # finish the previous chunk's softmax tails now, while the
                # DVE queue is shallow, so its o_proj groups (popped as
                # fillers below) never wait on the ao normalize
                while pending_tails and pending_tails[0][1] < tcx:
                    drain_tail(force=True)
                aos = [None] * HPC
                n_pt = 4 * tcx + 4
                # on the last chunk, keep a few groups back as PE cover for
                # the final softmax tail (GPSIMD reduce) in the epilogue
                reserve = 4 if (tcx + 1 == nchunk and tcx > 0) else 0

                for h in range(HPC):
                    in_head[0] = True
                    avp = av_ps.tile([128, CHUNK], F32, tag="av",
                                     name=f"av{tcx}_{h}")
                    sumA = work.tile([128, CHUNK], BF16, tag="psumA",
                                     bufs=3, name=f"psA{tcx}_{h}")
                    pts = [None] * n_pt
                    # diagonal key tile p (= 4*tcx + i) only reaches query
                    # columns >= 128*i: slice every op to the live columns
                    lo = [max(0, 128 * (p - 4 * tcx)) for p in range(n_pt)]
                    for p in range(n_pt):
                        sps = sc_ps.tile([128, CHUNK], F32, tag="sc")
                        nc.tensor.matmul(
                            sps[:, lo[p]:], kT_sb[:, 128 * p:128 * (p + 1)],
                            qT_heads[h][:, lo[p]:], start=True, stop=True)
                        # earlier heads' DVE tails are staged into this head's
                        # score stream once their GPSIMD reduce (~3.5us) has
                        # had time to complete
                        if p == 2:
                            drain_tail()
                        elif p > 2 and p % 4 == 2 and len(fillers) > reserve:
                            # mid-head filler: the exp (ACT) pipeline runs
                            # ~200ns/tile slower than the PE's two matmuls —
                            # give the PE independent work so it never laps
                            # the ACT queue (which also re-throttles the HAM
                            # clock gate)
                            fillers.pop(0)()
                        pt = ptpool.tile([128, CHUNK], BF16, tag="pt")
                        nc.scalar.activation(pt[:, lo[p]:], sps[:, lo[p]:],
                                             ACTF.Exp, scale=SCALE)
                        if p >= 4 * tcx:
                            i = p - 4 * tcx
                            nc.vector.tensor_tensor(
                                pt[:, 128 * i:128 * (i + 1)],
                                pt[:, 128 * i:128 * (i + 1)],
                                tri_sb[:], AL.mult)
                        if p == 0:
                            nc.vector.tensor_copy(sumA[:], pt[:])
                        else:
                            nc.vector.tensor_tensor(
                                sumA[:, lo[p]:], sumA[:, lo[p]:],
                                pt[:, lo[p]:], AL.add)
                        pts[p] = pt
                        if p > 2:
                            nc.tensor.matmul(
                                avp[:, lo[p - 3]:], v_tiles[p - 3][:],
                                pts[p - 3][:, lo[p - 3]:],
                                start=(p == 3), stop=False)
                    for pp in (n_pt - 3, n_pt - 2, n_pt - 1):
                        nc.tensor.matmul(
                            avp[:, lo[pp]:], v_tiles[pp][:],
                            pts[pp][:, lo[pp]:],
                            start=(pp == 0), stop=(pp == n_pt - 1))
                    tail_a(tcx, h, aos, avp, sumA)
                    # dependency-free filler keeps the PE busy while the
                    # ACT/DVE softmax pipeline of this head drains
                    n_fill = 2 if tcx + 1 == nchunk else 1
                    for _ in range(n_fill):
                        if len(fillers) > reserve:
                            fillers.pop(0)()
                in_head[0] = False
                for fi, f in enumerate(fillers):
                    f()
                    if fi % 2 == 1:
                        drain_tail()
                pending_oproj.append((tcx, aos))

            # last chunk's o_proj: drain the remaining softmax tails, then
            # emit the final groups (their ao inputs are ready by now)
            while pending_tails:
                drain_tail(force=True)
            for optcx, opaos in pending_oproj:
                for j in range(4):
                    for q in range(4):
                        emit_oproj_group(optcx, opaos, j, q)
    nc.finalize()
    return nc


_NC_CACHE = None


def _get_nc():
    global _NC_CACHE
    if _NC_CACHE is None:
        _NC_CACHE = build()
    return _NC_CACHE


_HALF_PERM = np.concatenate([np.arange(0, HD, 2), np.arange(1, HD, 2)])


def _etile(a):
    """[D, C] -> [128, (D//128)*C]: row p, block e holds original row
    128e+p, so each DMA packet is a long contiguous run."""
    d, c = a.shape
    return np.ascontiguousarray(
        a.reshape(d // 128, 128, c).transpose(1, 0, 2).reshape(128, -1))


def _xtile(a):
    """[D, T] -> [128, cols] laid out pair-major then e-major: block
    (P, e) holds rows 128e+p, cols [1024P:1024(P+1)] — long contiguous
    runs AND chunk-pair arrival granularity."""
    d, t = a.shape
    pw = min(1024, t)
    return np.ascontiguousarray(
        a.reshape(d // 128, 128, t // pw, pw).transpose(1, 2, 0, 3)
        .reshape(128, -1))


def _shard_inputs(x, wq, wk, wv, wo):
    perm_q = np.concatenate([128 * h + _HALF_PERM for h in range(HPC)])
    in_maps = []
    for c in range(N_CORES):
        b, k = c // 4, c % 4
        xT = _xtile(np.ascontiguousarray(x[b].T.astype(BF16NP)))
        wq_c = _etile(
            wq[:, QCOLS * k:QCOLS * (k + 1)][:, perm_q].astype(BF16NP))
        wk_c = _etile(
            wk[:, HD * k:HD * (k + 1)][:, _HALF_PERM].astype(BF16NP))
        wv_c = _etile(wv[:, HD * k:HD * (k + 1)].astype(BF16NP))
        wo_c = np.ascontiguousarray(
            wo[QCOLS * k:QCOLS * (k + 1), :].astype(BF16NP))
        in_maps.append({"xT": xT, "wq": wq_c, "wk": wk_c, "wv": wv_c,
                        "wo": wo_c})
    return in_maps


def kernel(x, wq, wk, wv, wo, _trace=False, _trace_kwargs=None):
    x, wq, wk, wv, wo = (np.asarray(a, dtype=np.float32)
                         for a in (x, wq, wk, wv, wo))
    nc = _get_nc()
    in_maps = _shard_inputs(x, wq, wk, wv, wo)
    kw = {}
    if _trace:
        kw = dict(trace=True, **(_trace_kwargs or {}))
    res = bass_utils.run_bass_kernel_spmd(
        nc, in_maps, list(range(N_CORES)), **kw)
    t = x.shape[1]
    out = np.empty((B, t, D), np.float32)
    for b in range(B):
        acc = res.results[4 * b]["out"].astype(np.float32)
        for k in range(1, 4):
            acc += res.results[4 * b + k]["out"].astype(np.float32)
        out[b] = acc
    kernel.last_result = res
    return out


# revision 52
# speedup vs baseline: 1.1682x; 1.1682x over previous
"""Distributed GQA attention kernel for 8 TRN2 NeuronCores.

Sharding: core c = 4*b + k handles batch b (of 2) and kv-head k (of 4),
i.e. Q heads 4k..4k+3 (column-parallel qkv).  Attention is computed per
core in transposed layout (S^T = K Q^T per 128-key tile, causal-skipped).
The output projection is ROW-parallel: each core contracts only its own
four heads' attention output against its 512 rows of wo, producing a
full [T, 2048] partial (bf16); the host sums the four partials of each
batch group.  No device collectives at all.

All matmuls run in bf16 (fp32 PSUM accumulation).  Softmax row sums are
accumulated as a bf16 partial-sum tile on the DVE; the partition
reduction + broadcast runs on the otherwise-idle GPSIMD engine
(partition_all_reduce, attn ucode library), so the PE never serializes
against the softmax tail.  Next-chunk projection chains and
previous-chunk o_proj groups are interleaved into the attention head
loop as PE filler so the TensorEngine never idles on the softmax
(ACT/DVE) pipeline.  o_proj groups keep the stationary operand (an
attention-output 128-column slice) across pairs of matmuls so weight
loads amortize.
"""
import sys
import numpy as np

for _p in ("/root/.axon_site", "/root/.axon_site/_ro/trn_rl_repo",
           "/root/.axon_site/_ro/pypackages"):
    if _p not in sys.path:
        sys.path.append(_p)

import ml_dtypes  # noqa: E402
import concourse.bass as bass  # noqa: E402
from concourse import bacc  # noqa: E402
import concourse.mybir as mybir  # noqa: E402
import concourse.bass_isa as bass_isa  # noqa: E402
from concourse import tile  # noqa: E402
from concourse import library_config  # noqa: E402
import concourse.bass_utils as bass_utils  # noqa: E402

F32 = mybir.dt.float32
BF16 = mybir.dt.bfloat16
AL = mybir.AluOpType
ACTF = mybir.ActivationFunctionType
BF16NP = ml_dtypes.bfloat16

B, T, D = 2, 2048, 2048
H, HK, HD = 16, 4, 128
HPC = 4                      # q-heads per core
QCOLS = HPC * HD             # 512 q columns per core
CHUNK = 512                  # t-chunk
NE = D // 128                # contraction e-chunks
THETA = 10000.0
SCALE = 1.0 / float(np.sqrt(HD))
N_CORES = 8


def _consts(t=T):
    freqs = 1.0 / THETA ** (np.arange(0, HD, 2, dtype=np.float64) / HD)
    pos = np.arange(t, dtype=np.float64)
    ang = np.outer(freqs, pos)                            # [64, t]
    cos = np.cos(ang).astype(np.float32)
    sin = np.sin(ang).astype(np.float32)
    cos_full = np.concatenate([cos, cos], axis=0).astype(BF16NP)  # [128, t]
    sin_pm = np.concatenate([-sin, sin], axis=0).astype(BF16NP)   # [128, t]
    swap = np.zeros((128, 128), np.float32)
    swap[(np.arange(128) + 64) % 128, np.arange(128)] = 1.0
    ident = np.eye(128, dtype=np.float32)
    # triangular causal mask for the 128-col strip where a diagonal key
    # tile crosses the query range: tri[pp, jj] = 1 iff pp <= jj
    tri = (np.arange(128)[:, None] <= np.arange(128)[None, :]).astype(
        np.float32)
    return (cos_full, sin_pm, swap.astype(BF16NP), ident.astype(BF16NP),
            tri.astype(BF16NP))


def build(t=T):
    nchunk = t // CHUNK
    npt = t // 128
    nc = bacc.Bacc("TRN2", target_bir_lowering=False, debug=False,
                   num_devices=N_CORES)
    # inputs arrive e-major-tiled ([128, NE*cols]: row p, block e holds
    # original row 128e+p) so every DMA moves >=2KB-contiguous runs: the
    # input phase is DMA packet-rate-bound, not byte-bound
    xT_e = nc.declare_dram_parameter("xT", [128, NE * t], BF16,
                                     isOutput=False)
    wq_e = nc.declare_dram_parameter("wq", [128, NE * QCOLS], BF16,
                                     isOutput=False)
    wk_e = nc.declare_dram_parameter("wk", [128, NE * HD], BF16,
                                     isOutput=False)
    wv_e = nc.declare_dram_parameter("wv", [128, NE * HD], BF16,
                                     isOutput=False)
    wo_e = nc.declare_dram_parameter("wo", [QCOLS, D], BF16, isOutput=False)
    out_e = nc.declare_dram_parameter("out", [t, D], BF16, isOutput=True)

    cos_np, sinpm_np, swap_np, ident_np, tri_np = _consts(t)
    cos_d = nc.inline_tensor(cos_np, "cos_c")
    sinpm_d = nc.inline_tensor(sinpm_np, "sinpm_c")
    swap_d = nc.inline_tensor(swap_np, "swap_c")
    ident_d = nc.inline_tensor(ident_np, "ident_c")
    tri_d = nc.inline_tensor(tri_np, "tri_c")

    with tile.TileContext(nc) as tc:
        with (
            tc.tile_pool(name="wpool", bufs=1) as wpool,
            tc.tile_pool(name="xpool", bufs=32) as xpool,
            tc.tile_pool(name="kvpool", bufs=1) as kvpool,
            tc.tile_pool(name="work", bufs=2) as work,
            tc.tile_pool(name="ptpool", bufs=8) as ptpool,
            tc.tile_pool(name="aopool", bufs=8) as aopool,
            tc.tile_pool(name="osbpool", bufs=2) as osbpool,
            tc.tile_pool(name="qkv_ps", bufs=2, space="PSUM") as qkv_ps,
            tc.tile_pool(name="sc_ps", bufs=2, space="PSUM") as sc_ps,
            tc.tile_pool(name="av_ps", bufs=1, space="PSUM") as av_ps,
            tc.tile_pool(name="sm_ps", bufs=1, space="PSUM") as sm_ps,
            tc.tile_pool(name="op_ps", bufs=2, space="PSUM") as op_ps,
        ):
            nc.gpsimd.load_library(library_config.attn)
            # ---- DMA plan: weights in e-group pieces (the first unlocks
            # the prologue after ~1us), full-T x tiles per e, then consts
            # and (lazily) wo.  Everything is >=1KB-contiguous per row.
            nq = max(1, NE // 4)
            pw = min(2 * CHUNK, t)          # chunk-pair width
            npair = t // pw
            wk_t = wpool.tile([128, NE * HD], BF16, name="wk_t")
            wv_t = wpool.tile([128, NE * HD], BF16, name="wv_t")
            wq_t = wpool.tile([128, NE * QCOLS], BF16, name="wq_t")
            xp = [[xpool.tile([128, pw], BF16, name=f"xe{P}_{e}", tag="xt")
                   for e in range(NE)] for P in range(npair)]
            half = NE * HD // 2
            nc.sync.dma_start(wk_t[:, 0:half], wk_e[:, 0:half])
            nc.sync.dma_start(wv_t[:, 0:half], wv_e[:, 0:half])
            for j in range(nq):
                qw = 4 * QCOLS
                nc.sync.dma_start(wq_t[:, qw * j:qw * (j + 1)],
                                  wq_e[:, qw * j:qw * (j + 1)])
                for e in range(4 * j, min(4 * (j + 1), NE)):
                    nc.sync.dma_start(xp[0][e][:],
                                      xT_e[:, pw * e:pw * (e + 1)])
                if j == 0:
                    nc.sync.dma_start(wk_t[:, half:], wk_e[:, half:])
                    nc.sync.dma_start(wv_t[:, half:], wv_e[:, half:])

            swap_sb = wpool.tile([128, 128], BF16, name="swap_sb")
            nc.sync.dma_start(swap_sb[:], swap_d[:, :])
            ident_sb = wpool.tile([128, 128], BF16, name="ident_sb")
            nc.sync.dma_start(ident_sb[:], ident_d[:, :])
            tri_sb = wpool.tile([128, 128], BF16, name="tri_sb")
            nc.sync.dma_start(tri_sb[:], tri_d[:, :])
            cos_sb = wpool.tile([128, t], BF16, name="cos_sb")
            nc.sync.dma_start(cos_sb[:], cos_d[:, :])
            sinpm_sb = wpool.tile([128, t], BF16, name="sinpm_sb")
            nc.sync.dma_start(sinpm_sb[:], sinpm_d[:, :])
            for P in range(1, npair):
                for e in range(NE):
                    base = NE * pw * P + pw * e
                    nc.sync.dma_start(xp[P][e][:],
                                      xT_e[:, base:base + pw])

            cpp = pw // CHUNK               # chunks per pair
            xts_by_chunk = {
                c: [xp[c // cpp][e][:, (c % cpp) * CHUNK:
                                    (c % cpp + 1) * CHUNK]
                    for e in range(NE)]
                for c in range(nchunk)}
            wo_sb = []

            kT_sb = kvpool.tile([128, t], BF16, name="kT_sb")
            v_tiles = [kvpool.tile([128, HD], BF16, name=f"v{i}")
                       for i in range(npt)]

            def rope_fin(qsb, out_ap, cols):
                """half-swap matmul + the two rotation multiplies"""
                qsw = sm_ps.tile([128, CHUNK], F32, tag="sm")
                nc.tensor.matmul(qsw[:], swap_sb[:], qsb[:],
                                 start=True, stop=True)
                t1 = work.tile([128, CHUNK], BF16, tag="ropeb")
                nc.vector.tensor_tensor(t1[:], qsb[:], cos_sb[:, cols], AL.mult)
                t2 = work.tile([128, CHUNK], BF16, tag="ropec")
                nc.vector.tensor_tensor(t2[:], qsw[:], sinpm_sb[:, cols],
                                        AL.mult)
                nc.vector.tensor_tensor(out_ap, t1[:], t2[:], AL.add)

            def load_wo():
                if not wo_sb:
                    for h in range(HPC):
                        wo_t = wpool.tile([128, D], BF16, name=f"wo{h}")
                        nc.sync.dma_start(wo_t[:],
                                          wo_e[128 * h:128 * (h + 1), :])
                        wo_sb.append(wo_t)

            osb_cur = {}
            in_head = [False]

            def emit_oproj_group(tcx, aos, j, q):
                """one [128 t-rows x 512 cols] block of the o_proj partial:
                accumulate the 4 heads; quarters aggregate into one
                [128, 2048] row-block so the store DMA moves 4KB rows."""
                while pending_tails and pending_tails[0][1] <= tcx:
                    drain_tail(force=True)
                ps = op_ps.tile([128, CHUNK], F32, tag="op")
                for h in range(HPC):
                    nc.tensor.matmul(ps[:], aos[h][:, 128 * j:128 * (j + 1)],
                                     wo_sb[h][:, CHUNK * q:CHUNK * (q + 1)],
                                     start=(h == 0), stop=(h == HPC - 1))
                if q == 0:
                    osb_cur[(tcx, j)] = osbpool.tile(
                        [128, D], BF16, tag="osb", name=f"osb{tcx}_{j}")
                osb = osb_cur[(tcx, j)]
                # alternate the drain between ACT and DVE so neither queue
                # backs up in front of the attention pipeline
                if (j + q) % 2 == 0:
                    nc.scalar.copy(osb[:, CHUNK * q:CHUNK * (q + 1)], ps[:])
                else:
                    nc.vector.tensor_copy(osb[:, CHUNK * q:CHUNK * (q + 1)],
                                          ps[:])
                if q == 3:
                    row = tcx * CHUNK + 128 * j
                    nc.sync.dma_start(out_e[row:row + 128, :], osb[:])
                    del osb_cur[(tcx, j)]

            def chain_mm(ps, kind, h, xts_c, e):
                if kind == "q":
                    base = QCOLS * e + 128 * h
                    w_ap = wq_t[:, base:base + 128]
                elif kind == "k":
                    w_ap = wk_t[:, HD * e:HD * (e + 1)]
                else:
                    w_ap = wv_t[:, HD * e:HD * (e + 1)]
                nc.tensor.matmul(ps[:], w_ap, xts_c[e],
                                 start=(e == 0), stop=(e == NE - 1))

            def chain_copy(ps, kind):
                """PSUM -> bf16 SBUF drain of a finished chain (frees the
                chain bank; the PE-side finish is deferred)."""
                tag = "vsb" if kind == "v" else "ropea"
                sb = work.tile([128, CHUNK], BF16, tag=tag, bufs=6)
                nc.vector.tensor_copy(sb[:], ps[:])
                return sb

            def chain_fin(cidx, kind, h, sb, qT_list):
                """deferred PE-side finish: rope (q/k) or transposes (v).
                Runs a few filler slots after the chain so its DVE drain has
                cleared the queue and the PE never waits on it."""
                ccols = slice(cidx * CHUNK, (cidx + 1) * CHUNK)
                if kind == "v":
                    for j in range(4):
                        tp = sm_ps.tile([128, 128], BF16, tag="sm")
                        nc.tensor.transpose(
                            tp[:], sb[:, 128 * j:128 * (j + 1)],
                            ident_sb[:])
                        nc.vector.tensor_copy(
                            v_tiles[4 * cidx + j][:], tp[:])
                elif kind == "q":
                    qT = work.tile([128, CHUNK], BF16, tag="qT",
                                   bufs=10, name=f"qT{cidx}_{h}")
                    qT_list[h] = qT
                    rope_fin(sb, qT[:], ccols)
                else:
                    rope_fin(sb, kT_sb[:, ccols], ccols)

            def emit_chain(cidx, kind, h, xts_c, qT_list, defer=None):
                """one projection accumulation chain; the rope/transpose
                finish is appended to `defer` (or emitted inline)."""
                ps = qkv_ps.tile([128, CHUNK], F32, tag="qkv",
                                 name=f"ps_{cidx}_{kind}{h}")
                for e in range(NE):
                    chain_mm(ps, kind, h, xts_c, e)
                sb = chain_copy(ps, kind)
                fin = lambda: chain_fin(cidx, kind, h, sb, qT_list)  # noqa
                if defer is None:
                    fin()
                else:
                    defer.append(fin)

            CHAIN_ORDER = [("k", 0), ("v", 0)] + [("q", h) for h in range(HPC)]
            pending_oproj = []
            pending_tails = []
            ghead = [0]
            qT_next = [None] * HPC

            def tail_a(tcx, h, aos, avp, sumA):
                """partition-reduce + broadcast the softmax row sums on the
                GPSIMD engine; free the AV PSUM bank with an ACT copy."""
                sums_bc = work.tile([128, CHUNK], F32, tag="sumbc",
                                    bufs=4, name=f"sbc{tcx}_{h}")
                nc.gpsimd.partition_all_reduce(
                    sums_bc[:], sumA[:], channels=128,
                    reduce_op=bass_isa.ReduceOp.add)
                av_sb = work.tile([128, CHUNK], BF16, tag="avsb",
                                  bufs=4, name=f"avsb{tcx}_{h}")
                nc.scalar.copy(av_sb[:], avp[:])
                pending_tails.append((ghead[0], tcx, h, aos, av_sb, sums_bc))
                ghead[0] += 1

            def tail_b(_idx, tcx, h, aos, av_sb, sums_bc):
                """reciprocal + normalize into the o_proj input tile"""
                recb = work.tile([128, CHUNK], F32, tag="recb",
                                 name=f"recb{tcx}_{h}")
                nc.vector.reciprocal_approx_fast(recb[:], sums_bc[:])
                ao = aopool.tile([128, CHUNK], BF16, tag="ao",
                                 name=f"ao{tcx}_{h}")
                nc.vector.tensor_tensor(ao[:], av_sb[:], recb[:], AL.mult)
                aos[h] = ao

            def drain_tail(min_age=2, force=False):
                if pending_tails and (
                        force or pending_tails[0][0] <= ghead[0] - min_age):
                    tail_b(*pending_tails.pop(0))
            # prologue: all six chunk-0 projection chains run e-major on
            # borrowed (idle) PSUM banks so each arriving DMA group unlocks
            # six matmuls and the PE paces with DMA arrival.  K/Q0/V finish
            # inline (chunk-0 head 0 needs them); Q1-Q3 finishes become the
            # first fillers of chunk 0.
            pro = [("k", 0, qkv_ps.tile([128, CHUNK], F32, tag="qkv",
                                        name="ps_pro_k")),
                   ("q", 0, qkv_ps.tile([128, CHUNK], F32, tag="qkv",
                                        name="ps_pro_q0")),
                   ("v", 0, sc_ps.tile([128, CHUNK], F32, tag="sc",
                                       name="ps_pro_v")),
                   ("q", 1, sc_ps.tile([128, CHUNK], F32, tag="sc",
                                       name="ps_pro_q1")),
                   ("q", 2, op_ps.tile([128, CHUNK], F32, tag="op",
                                       name="ps_pro_q2")),
                   ("q", 3, op_ps.tile([128, CHUNK], F32, tag="op",
                                       name="ps_pro_q3"))]
            for e in range(NE):
                for kind, h, ps in pro:
                    chain_mm(ps, kind, h, xts_by_chunk[0], e)
            pro_sb = {(kind, h): chain_copy(ps, kind)
                      for kind, h, ps in pro}
            pro_fins = []
            for kind, h in (("k", 0), ("q", 0), ("v", 0)):
                chain_fin(0, kind, h, pro_sb[(kind, h)], qT_next)
            for kind, h in (("q", 1), ("q", 2), ("q", 3)):
                def mk_fin(kk=kind, hh=h, ql=qT_next):
                    return lambda: chain_fin(0, kk, hh, pro_sb[(kk, hh)], ql)
                pro_fins.append(mk_fin())

            for tcx in range(nchunk):
                qT_heads = qT_next
                # filler work: PE instructions with no dependence on this
                # chunk's softmax pipeline, spread between attention heads
                fillers = pro_fins
                pro_fins = []
                if tcx + 1 < nchunk:
                    qT_next = [None] * HPC
                    qn = qT_next

                    def mk_chain(kind, ch, xc=xts_by_chunk[tcx + 1], qq=qn,
                                 ci=tcx + 1):
                        return lambda: emit_chain(ci, kind, ch, xc, qq,
                                                  defer=fillers)

                    fillers += [mk_chain(kind, ch) for kind, ch in CHAIN_ORDER]
                while pending_oproj:
                    optcx, opaos = pending_oproj.pop(0)

                    def mk_grp(j, q, ti=optcx, aa=opaos):
                        return lambda: emit_oproj_group(ti, aa, j, q)

                    fillers += [mk_grp(j, q)
                                for j in range(4) for q in range(4)]

                # ---- attention for q-chunk tcx ----
                load_wo()
                # finish the previous chunk's softmax tails now, while the
                # DVE queue is shallow, so its o_proj groups (popped as
                # fillers below) never wait on the ao normalize
                while pending_tails and pending_tails[0][1] < tcx:
                    drain_tail(force=True)
                aos = [None] * HPC
                n_pt = 4 * tcx + 4
                # on the last chunk, keep a few groups back as PE cover for
                # the final softmax tail (GPSIMD reduce) in the epilogue
                reserve = 4 if (tcx + 1 == nchunk and tcx > 0) else 0

                for h in range(HPC):
                    in_head[0] = True
                    avp = av_ps.tile([128, CHUNK], F32, tag="av",
                                     name=f"av{tcx}_{h}")
                    sumA = work.tile([128, CHUNK], BF16, tag="psumA",
                                     bufs=3, name=f"psA{tcx}_{h}")
                    pts = [None] * n_pt
                    # diagonal key tile p (= 4*tcx + i) only reaches query
                    # columns >= 128*i: slice every op to the live columns
                    lo = [max(0, 128 * (p - 4 * tcx)) for p in range(n_pt)]
                    for p in range(n_pt):
                        sps = sc_ps.tile([128, CHUNK], F32, tag="sc")
                        nc.tensor.matmul(
                            sps[:, lo[p]:], kT_sb[:, 128 * p:128 * (p + 1)],
                            qT_heads[h][:, lo[p]:], start=True, stop=True)
                        # earlier heads' DVE tails are staged into this head's
                        # score stream once their GPSIMD reduce (~3.5us) has
                        # had time to complete
                        if p == 2:
                            drain_tail()
                        elif p > 2 and p % 4 == 2 and len(fillers) > reserve:
                            # mid-head filler: the exp (ACT) pipeline runs
                            # ~200ns/tile slower than the PE's two matmuls —
                            # give the PE independent work so it never laps
                            # the ACT queue (which also re-throttles the HAM
                            # clock gate)
                            fillers.pop(0)()
                        pt = ptpool.tile([128, CHUNK], BF16, tag="pt")
                        nc.scalar.activation(pt[:, lo[p]:], sps[:, lo[p]:],
                                             ACTF.Exp, scale=SCALE)
                        if p >= 4 * tcx:
                            i = p - 4 * tcx
                            nc.vector.tensor_tensor(
                                pt[:, 128 * i:128 * (i + 1)],
                                pt[:, 128 * i:128 * (i + 1)],
                                tri_sb[:], AL.mult)
                        if p == 0:
                            nc.vector.tensor_copy(sumA[:], pt[:])
                        else:
                            nc.vector.tensor_tensor(
                                sumA[:, lo[p]:], sumA[:, lo[p]:],
                                pt[:, lo[p]:], AL.add)
                        pts[p] = pt
                        if p > 1:
                            nc.tensor.matmul(
                                avp[:, lo[p - 2]:], v_tiles[p - 2][:],
                                pts[p - 2][:, lo[p - 2]:],
                                start=(p == 2), stop=False)
                    for pp in (n_pt - 2, n_pt - 1):
                        nc.tensor.matmul(
                            avp[:, lo[pp]:], v_tiles[pp][:],
                            pts[pp][:, lo[pp]:],
                            start=(pp == 0), stop=(pp == n_pt - 1))
                    tail_a(tcx, h, aos, avp, sumA)
                    # dependency-free filler keeps the PE busy while the
                    # ACT/DVE softmax pipeline of this head drains
                    n_fill = 2 if tcx + 1 == nchunk else 1
                    for _ in range(n_fill):
                        if len(fillers) > reserve:
                            fillers.pop(0)()
                in_head[0] = False
                for fi, f in enumerate(fillers):
                    f()
                    if fi % 2 == 1:
                        drain_tail()
                pending_oproj.append((tcx, aos))

            # last chunk's o_proj: drain the remaining softmax tails, then
            # emit the final groups (their ao inputs are ready by now)
            while pending_tails:
                drain_tail(force=True)
            for optcx, opaos in pending_oproj:
                for j in range(4):
                    for q in range(4):
                        emit_oproj_group(optcx, opaos, j, q)
    nc.finalize()
    return nc


_NC_CACHE = None


def _get_nc():
    global _NC_CACHE
    if _NC_CACHE is None:
        _NC_CACHE = build()
    return _NC_CACHE


_HALF_PERM = np.concatenate([np.arange(0, HD, 2), np.arange(1, HD, 2)])


def _etile(a):
    """[D, C] -> [128, (D//128)*C]: row p, block e holds original row
    128e+p, so each DMA packet is a long contiguous run."""
    d, c = a.shape
    return np.ascontiguousarray(
        a.reshape(d // 128, 128, c).transpose(1, 0, 2).reshape(128, -1))


def _xtile(a):
    """[D, T] -> [128, cols] laid out pair-major then e-major: block
    (P, e) holds rows 128e+p, cols [1024P:1024(P+1)] — long contiguous
    runs AND chunk-pair arrival granularity."""
    d, t = a.shape
    pw = min(1024, t)
    return np.ascontiguousarray(
        a.reshape(d // 128, 128, t // pw, pw).transpose(1, 2, 0, 3)
        .reshape(128, -1))


def _shard_inputs(x, wq, wk, wv, wo):
    perm_q = np.concatenate([128 * h + _HALF_PERM for h in range(HPC)])
    in_maps = []
    for c in range(N_CORES):
        b, k = c // 4, c % 4
        xT = _xtile(np.ascontiguousarray(x[b].T.astype(BF16NP)))
        wq_c = _etile(
            wq[:, QCOLS * k:QCOLS * (k + 1)][:, perm_q].astype(BF16NP))
        wk_c = _etile(
            wk[:, HD * k:HD * (k + 1)][:, _HALF_PERM].astype(BF16NP))
        wv_c = _etile(wv[:, HD * k:HD * (k + 1)].astype(BF16NP))
        wo_c = np.ascontiguousarray(
            wo[QCOLS * k:QCOLS * (k + 1), :].astype(BF16NP))
        in_maps.append({"xT": xT, "wq": wq_c, "wk": wk_c, "wv": wv_c,
                        "wo": wo_c})
    return in_maps


def kernel(x, wq, wk, wv, wo, _trace=False, _trace_kwargs=None):
    x, wq, wk, wv, wo = (np.asarray(a, dtype=np.float32)
                         for a in (x, wq, wk, wv, wo))
    nc = _get_nc()
    in_maps = _shard_inputs(x, wq, wk, wv, wo)
    kw = {}
    if _trace:
        kw = dict(trace=True, **(_trace_kwargs or {}))
    res = bass_utils.run_bass_kernel_spmd(
        nc, in_maps, list(range(N_CORES)), **kw)
    t = x.shape[1]
    out = np.empty((B, t, D), np.float32)
    for b in range(B):
        acc = res.results[4 * b]["out"].astype(np.float32)
        for k in range(1, 4):
            acc += res.results[4 * b + k]["out"].astype(np.float32)
        out[b] = acc
    kernel.last_result = res
    return out


# revision 55
# speedup vs baseline: 1.1704x; 1.0019x over previous
"""Distributed GQA attention kernel for 8 TRN2 NeuronCores.

Sharding: core c = 4*b + k handles batch b (of 2) and kv-head k (of 4),
i.e. Q heads 4k..4k+3 (column-parallel qkv).  Attention is computed per
core in transposed layout (S^T = K Q^T per 128-key tile, causal-skipped).
The output projection is ROW-parallel: each core contracts only its own
four heads' attention output against its 512 rows of wo, producing a
full [T, 2048] partial (bf16); the host sums the four partials of each
batch group.  No device collectives at all.

All matmuls run in bf16 (fp32 PSUM accumulation).  Softmax row sums are
accumulated as a bf16 partial-sum tile on the DVE; the partition
reduction + broadcast runs on the otherwise-idle GPSIMD engine
(partition_all_reduce, attn ucode library), so the PE never serializes
against the softmax tail.  Next-chunk projection chains and
previous-chunk o_proj groups are interleaved into the attention head
loop as PE filler so the TensorEngine never idles on the softmax
(ACT/DVE) pipeline.  o_proj groups keep the stationary operand (an
attention-output 128-column slice) across pairs of matmuls so weight
loads amortize.
"""
import sys
import numpy as np

for _p in ("/root/.axon_site", "/root/.axon_site/_ro/trn_rl_repo",
           "/root/.axon_site/_ro/pypackages"):
    if _p not in sys.path:
        sys.path.append(_p)

import ml_dtypes  # noqa: E402
import concourse.bass as bass  # noqa: E402
from concourse import bacc  # noqa: E402
import concourse.mybir as mybir  # noqa: E402
import concourse.bass_isa as bass_isa  # noqa: E402
from concourse import tile  # noqa: E402
from concourse import library_config  # noqa: E402
import concourse.bass_utils as bass_utils  # noqa: E402

F32 = mybir.dt.float32
BF16 = mybir.dt.bfloat16
AL = mybir.AluOpType
ACTF = mybir.ActivationFunctionType
BF16NP = ml_dtypes.bfloat16

B, T, D = 2, 2048, 2048
H, HK, HD = 16, 4, 128
HPC = 4                      # q-heads per core
QCOLS = HPC * HD             # 512 q columns per core
CHUNK = 512                  # t-chunk
NE = D // 128                # contraction e-chunks
THETA = 10000.0
SCALE = 1.0 / float(np.sqrt(HD))
N_CORES = 8


def _consts(t=T):
    freqs = 1.0 / THETA ** (np.arange(0, HD, 2, dtype=np.float64) / HD)
    pos = np.arange(t, dtype=np.float64)
    ang = np.outer(freqs, pos)                            # [64, t]
    cos = np.cos(ang).astype(np.float32)
    sin = np.sin(ang).astype(np.float32)
    cos_full = np.concatenate([cos, cos], axis=0).astype(BF16NP)  # [128, t]
    sin_pm = np.concatenate([-sin, sin], axis=0).astype(BF16NP)   # [128, t]
    swap = np.zeros((128, 128), np.float32)
    swap[(np.arange(128) + 64) % 128, np.arange(128)] = 1.0
    ident = np.eye(128, dtype=np.float32)
    # triangular causal mask for the 128-col strip where a diagonal key
    # tile crosses the query range: tri[pp, jj] = 1 iff pp <= jj
    tri = (np.arange(128)[:, None] <= np.arange(128)[None, :]).astype(
        np.float32)
    return (cos_full, sin_pm, swap.astype(BF16NP), ident.astype(BF16NP),
            tri.astype(BF16NP))


def build(t=T):
    nchunk = t // CHUNK
    npt = t // 128
    nc = bacc.Bacc("TRN2", target_bir_lowering=False, debug=False,
                   num_devices=N_CORES)
    # inputs arrive e-major-tiled ([128, NE*cols]: row p, block e holds
    # original row 128e+p) so every DMA moves >=2KB-contiguous runs: the
    # input phase is DMA packet-rate-bound, not byte-bound
    xT_e = nc.declare_dram_parameter("xT", [128, NE * t], BF16,
                                     isOutput=False)
    wq_e = nc.declare_dram_parameter("wq", [128, NE * QCOLS], BF16,
                                     isOutput=False)
    wk_e = nc.declare_dram_parameter("wk", [128, NE * HD], BF16,
                                     isOutput=False)
    wv_e = nc.declare_dram_parameter("wv", [128, NE * HD], BF16,
                                     isOutput=False)
    wo_e = nc.declare_dram_parameter("wo", [QCOLS, D], BF16, isOutput=False)
    out_e = nc.declare_dram_parameter("out", [t, D], BF16, isOutput=True)

    cos_np, sinpm_np, swap_np, ident_np, tri_np = _consts(t)
    cos_d = nc.inline_tensor(cos_np, "cos_c")
    sinpm_d = nc.inline_tensor(sinpm_np, "sinpm_c")
    swap_d = nc.inline_tensor(swap_np, "swap_c")
    ident_d = nc.inline_tensor(ident_np, "ident_c")
    tri_d = nc.inline_tensor(tri_np, "tri_c")

    with tile.TileContext(nc) as tc:
        with (
            tc.tile_pool(name="wpool", bufs=1) as wpool,
            tc.tile_pool(name="xpool", bufs=32) as xpool,
            tc.tile_pool(name="kvpool", bufs=1) as kvpool,
            tc.tile_pool(name="work", bufs=2) as work,
            tc.tile_pool(name="ptpool", bufs=8) as ptpool,
            tc.tile_pool(name="aopool", bufs=8) as aopool,
            tc.tile_pool(name="osbpool", bufs=2) as osbpool,
            tc.tile_pool(name="qkv_ps", bufs=2, space="PSUM") as qkv_ps,
            tc.tile_pool(name="sc_ps", bufs=2, space="PSUM") as sc_ps,
            tc.tile_pool(name="av_ps", bufs=1, space="PSUM") as av_ps,
            tc.tile_pool(name="sm_ps", bufs=1, space="PSUM") as sm_ps,
            tc.tile_pool(name="op_ps", bufs=2, space="PSUM") as op_ps,
        ):
            nc.gpsimd.load_library(library_config.attn)
            # ---- DMA plan: weights in e-group pieces (the first unlocks
            # the prologue after ~1us), full-T x tiles per e, then consts
            # and (lazily) wo.  Everything is >=1KB-contiguous per row.
            nq = max(1, NE // 4)
            pw = min(2 * CHUNK, t)          # chunk-pair width
            npair = t // pw
            wk_t = wpool.tile([128, NE * HD], BF16, name="wk_t")
            wv_t = wpool.tile([128, NE * HD], BF16, name="wv_t")
            wq_t = wpool.tile([128, NE * QCOLS], BF16, name="wq_t")
            xp = [[xpool.tile([128, pw], BF16, name=f"xe{P}_{e}", tag="xt")
                   for e in range(NE)] for P in range(npair)]
            half = NE * HD // 2
            nc.sync.dma_start(wk_t[:, 0:half], wk_e[:, 0:half])
            nc.sync.dma_start(wv_t[:, 0:half], wv_e[:, 0:half])
            for j in range(nq):
                qw = 4 * QCOLS
                nc.sync.dma_start(wq_t[:, qw * j:qw * (j + 1)],
                                  wq_e[:, qw * j:qw * (j + 1)])
                for e in range(4 * j, min(4 * (j + 1), NE)):
                    nc.sync.dma_start(xp[0][e][:],
                                      xT_e[:, pw * e:pw * (e + 1)])
                if j == 0:
                    nc.sync.dma_start(wk_t[:, half:], wk_e[:, half:])
                    nc.sync.dma_start(wv_t[:, half:], wv_e[:, half:])

            swap_sb = wpool.tile([128, 128], BF16, name="swap_sb")
            nc.sync.dma_start(swap_sb[:], swap_d[:, :])
            ident_sb = wpool.tile([128, 128], BF16, name="ident_sb")
            nc.sync.dma_start(ident_sb[:], ident_d[:, :])
            tri_sb = wpool.tile([128, 128], BF16, name="tri_sb")
            nc.sync.dma_start(tri_sb[:], tri_d[:, :])
            cos_sb = wpool.tile([128, t], BF16, name="cos_sb")
            nc.sync.dma_start(cos_sb[:], cos_d[:, :])
            sinpm_sb = wpool.tile([128, t], BF16, name="sinpm_sb")
            nc.sync.dma_start(sinpm_sb[:], sinpm_d[:, :])
            for P in range(1, npair):
                for e in range(NE):
                    base = NE * pw * P + pw * e
                    nc.sync.dma_start(xp[P][e][:],
                                      xT_e[:, base:base + pw])

            cpp = pw // CHUNK               # chunks per pair
            xts_by_chunk = {
                c: [xp[c // cpp][e][:, (c % cpp) * CHUNK:
                                    (c % cpp + 1) * CHUNK]
                    for e in range(NE)]
                for c in range(nchunk)}
            wo_sb = []

            kT_sb = kvpool.tile([128, t], BF16, name="kT_sb")
            v_tiles = [kvpool.tile([128, HD], BF16, name=f"v{i}")
                       for i in range(npt)]

            def rope_fin(qsb, out_ap, cols):
                """half-swap matmul + the two rotation multiplies"""
                qsw = sm_ps.tile([128, CHUNK], F32, tag="sm")
                nc.tensor.matmul(qsw[:], swap_sb[:], qsb[:],
                                 start=True, stop=True)
                t1 = work.tile([128, CHUNK], BF16, tag="ropeb")
                nc.vector.tensor_tensor(t1[:], qsb[:], cos_sb[:, cols], AL.mult)
                t2 = work.tile([128, CHUNK], BF16, tag="ropec")
                nc.vector.tensor_tensor(t2[:], qsw[:], sinpm_sb[:, cols],
                                        AL.mult)
                nc.vector.tensor_tensor(out_ap, t1[:], t2[:], AL.add)

            def load_wo():
                if not wo_sb:
                    for h in range(HPC):
                        wo_t = wpool.tile([128, D], BF16, name=f"wo{h}")
                        nc.sync.dma_start(wo_t[:],
                                          wo_e[128 * h:128 * (h + 1), :])
                        wo_sb.append(wo_t)

            osb_cur = {}
            in_head = [False]

            def emit_oproj_group(tcx, aos, j, q):
                """one [128 t-rows x 512 cols] block of the o_proj partial:
                accumulate the 4 heads; quarters aggregate into one
                [128, 2048] row-block so the store DMA moves 4KB rows."""
                while pending_tails and pending_tails[0][1] <= tcx:
                    drain_tail(force=True)
                ps = op_ps.tile([128, CHUNK], F32, tag="op")
                for h in range(HPC):
                    nc.tensor.matmul(ps[:], aos[h][:, 128 * j:128 * (j + 1)],
                                     wo_sb[h][:, CHUNK * q:CHUNK * (q + 1)],
                                     start=(h == 0), stop=(h == HPC - 1))
                if q == 0:
                    osb_cur[(tcx, j)] = osbpool.tile(
                        [128, D], BF16, tag="osb", name=f"osb{tcx}_{j}")
                osb = osb_cur[(tcx, j)]
                # alternate the drain between ACT and DVE so neither queue
                # backs up in front of the attention pipeline
                if (j + q) % 2 == 0:
                    nc.scalar.copy(osb[:, CHUNK * q:CHUNK * (q + 1)], ps[:])
                else:
                    nc.vector.tensor_copy(osb[:, CHUNK * q:CHUNK * (q + 1)],
                                          ps[:])
                if q == 3:
                    row = tcx * CHUNK + 128 * j
                    nc.sync.dma_start(out_e[row:row + 128, :], osb[:])
                    del osb_cur[(tcx, j)]

            def chain_mm(ps, kind, h, xts_c, e):
                if kind == "q":
                    base = QCOLS * e + 128 * h
                    w_ap = wq_t[:, base:base + 128]
                elif kind == "k":
                    w_ap = wk_t[:, HD * e:HD * (e + 1)]
                else:
                    w_ap = wv_t[:, HD * e:HD * (e + 1)]
                nc.tensor.matmul(ps[:], w_ap, xts_c[e],
                                 start=(e == 0), stop=(e == NE - 1))

            def chain_copy(ps, kind):
                """PSUM -> bf16 SBUF drain of a finished chain (frees the
                chain bank; the PE-side finish is deferred)."""
                tag = "vsb" if kind == "v" else "ropea"
                sb = work.tile([128, CHUNK], BF16, tag=tag, bufs=6)
                nc.vector.tensor_copy(sb[:], ps[:])
                return sb

            def chain_fin(cidx, kind, h, sb, qT_list):
                """deferred PE-side finish: rope (q/k) or transposes (v).
                Runs a few filler slots after the chain so its DVE drain has
                cleared the queue and the PE never waits on it."""
                ccols = slice(cidx * CHUNK, (cidx + 1) * CHUNK)
                if kind == "v":
                    for j in range(4):
                        tp = sm_ps.tile([128, 128], BF16, tag="sm")
                        nc.tensor.transpose(
                            tp[:], sb[:, 128 * j:128 * (j + 1)],
                            ident_sb[:])
                        nc.vector.tensor_copy(
                            v_tiles[4 * cidx + j][:], tp[:])
                elif kind == "q":
                    qT = work.tile([128, CHUNK], BF16, tag="qT",
                                   bufs=10, name=f"qT{cidx}_{h}")
                    qT_list[h] = qT
                    rope_fin(sb, qT[:], ccols)
                else:
                    rope_fin(sb, kT_sb[:, ccols], ccols)

            def emit_chain(cidx, kind, h, xts_c, qT_list, defer=None):
                """one projection accumulation chain; the rope/transpose
                finish is appended to `defer` (or emitted inline)."""
                ps = qkv_ps.tile([128, CHUNK], F32, tag="qkv",
                                 name=f"ps_{cidx}_{kind}{h}")
                for e in range(NE):
                    chain_mm(ps, kind, h, xts_c, e)
                sb = chain_copy(ps, kind)
                fin = lambda: chain_fin(cidx, kind, h, sb, qT_list)  # noqa
                if defer is None:
                    fin()
                else:
                    defer.append(fin)

            CHAIN_ORDER = [("k", 0), ("v", 0)] + [("q", h) for h in range(HPC)]
            pending_oproj = []
            pending_tails = []
            ghead = [0]
            qT_next = [None] * HPC

            def tail_a(tcx, h, aos, avp, sumA):
                """partition-reduce + broadcast the softmax row sums on the
                GPSIMD engine; free the AV PSUM bank with an ACT copy."""
                sums_bc = work.tile([128, CHUNK], F32, tag="sumbc",
                                    bufs=4, name=f"sbc{tcx}_{h}")
                nc.gpsimd.partition_all_reduce(
                    sums_bc[:], sumA[:], channels=128,
                    reduce_op=bass_isa.ReduceOp.add)
                av_sb = work.tile([128, CHUNK], BF16, tag="avsb",
                                  bufs=4, name=f"avsb{tcx}_{h}")
                nc.scalar.copy(av_sb[:], avp[:])
                pending_tails.append((ghead[0], tcx, h, aos, av_sb, sums_bc))
                ghead[0] += 1

            def tail_b(_idx, tcx, h, aos, av_sb, sums_bc):
                """reciprocal + normalize into the o_proj input tile"""
                recb = work.tile([128, CHUNK], F32, tag="recb",
                                 name=f"recb{tcx}_{h}")
                nc.vector.reciprocal_approx_fast(recb[:], sums_bc[:])
                ao = aopool.tile([128, CHUNK], BF16, tag="ao",
                                 name=f"ao{tcx}_{h}")
                nc.vector.tensor_tensor(ao[:], av_sb[:], recb[:], AL.mult)
                aos[h] = ao

            def drain_tail(min_age=2, force=False):
                if pending_tails and (
                        force or pending_tails[0][0] <= ghead[0] - min_age):
                    tail_b(*pending_tails.pop(0))
            # prologue: all six chunk-0 projection chains run e-major on
            # borrowed (idle) PSUM banks so each arriving DMA group unlocks
            # six matmuls and the PE paces with DMA arrival.  K/Q0/V finish
            # inline (chunk-0 head 0 needs them); Q1-Q3 finishes become the
            # first fillers of chunk 0.
            pro = [("k", 0, qkv_ps.tile([128, CHUNK], F32, tag="qkv",
                                        name="ps_pro_k")),
                   ("q", 0, qkv_ps.tile([128, CHUNK], F32, tag="qkv",
                                        name="ps_pro_q0")),
                   ("v", 0, sc_ps.tile([128, CHUNK], F32, tag="sc",
                                       name="ps_pro_v")),
                   ("q", 1, sc_ps.tile([128, CHUNK], F32, tag="sc",
                                       name="ps_pro_q1")),
                   ("q", 2, op_ps.tile([128, CHUNK], F32, tag="op",
                                       name="ps_pro_q2")),
                   ("q", 3, op_ps.tile([128, CHUNK], F32, tag="op",
                                       name="ps_pro_q3"))]
            for e in range(NE):
                for kind, h, ps in pro:
                    chain_mm(ps, kind, h, xts_by_chunk[0], e)
            pro_sb = {(kind, h): chain_copy(ps, kind)
                      for kind, h, ps in pro}
            pro_fins = []
            for kind, h in (("k", 0), ("q", 0), ("v", 0)):
                chain_fin(0, kind, h, pro_sb[(kind, h)], qT_next)
            for kind, h in (("q", 1), ("q", 2), ("q", 3)):
                def mk_fin(kk=kind, hh=h, ql=qT_next):
                    return lambda: chain_fin(0, kk, hh, pro_sb[(kk, hh)], ql)
                pro_fins.append(mk_fin())

            for tcx in range(nchunk):
                qT_heads = qT_next
                # filler work: PE instructions with no dependence on this
                # chunk's softmax pipeline, spread between attention heads
                fillers = pro_fins
                pro_fins = []
                if tcx + 1 < nchunk:
                    qT_next = [None] * HPC
                    qn = qT_next

                    def mk_chain(kind, ch, xc=xts_by_chunk[tcx + 1], qq=qn,
                                 ci=tcx + 1):
                        return lambda: emit_chain(ci, kind, ch, xc, qq,
                                                  defer=fillers)

                    fillers += [mk_chain(kind, ch) for kind, ch in CHAIN_ORDER]
                while pending_oproj:
                    optcx, opaos = pending_oproj.pop(0)

                    def mk_grp(j, q, ti=optcx, aa=opaos):
                        return lambda: emit_oproj_group(ti, aa, j, q)

                    fillers += [mk_grp(j, q)
                                for j in range(4) for q in range(4)]

                # ---- attention for q-chunk tcx ----
                load_wo()
                # finish the previous chunk's softmax tails now, while the
                # DVE queue is shallow, so its o_proj groups (popped as
                # fillers below) never wait on the ao normalize
                while pending_tails and pending_tails[0][1] < tcx:
                    drain_tail(force=True)
                aos = [None] * HPC
                n_pt = 4 * tcx + 4
                # on the last chunk, keep a few groups back as PE cover for
                # the final softmax tail (GPSIMD reduce) in the epilogue
                reserve = 6 if (tcx + 1 == nchunk and tcx > 0) else 0

                for h in range(HPC):
                    in_head[0] = True
                    avp = av_ps.tile([128, CHUNK], F32, tag="av",
                                     name=f"av{tcx}_{h}")
                    sumA = work.tile([128, CHUNK], BF16, tag="psumA",
                                     bufs=3, name=f"psA{tcx}_{h}")
                    pts = [None] * n_pt
                    # diagonal key tile p (= 4*tcx + i) only reaches query
                    # columns >= 128*i: slice every op to the live columns
                    lo = [max(0, 128 * (p - 4 * tcx)) for p in range(n_pt)]
                    for p in range(n_pt):
                        sps = sc_ps.tile([128, CHUNK], F32, tag="sc")
                        nc.tensor.matmul(
                            sps[:, lo[p]:], kT_sb[:, 128 * p:128 * (p + 1)],
                            qT_heads[h][:, lo[p]:], start=True, stop=True)
                        # earlier heads' DVE tails are staged into this head's
                        # score stream once their GPSIMD reduce (~3.5us) has
                        # had time to complete
                        if p == 2:
                            drain_tail()
                        elif p > 2 and p % 4 == 2 and len(fillers) > reserve:
                            # mid-head filler: the exp (ACT) pipeline runs
                            # ~200ns/tile slower than the PE's two matmuls —
                            # give the PE independent work so it never laps
                            # the ACT queue (which also re-throttles the HAM
                            # clock gate)
                            fillers.pop(0)()
                        pt = ptpool.tile([128, CHUNK], BF16, tag="pt")
                        nc.scalar.activation(pt[:, lo[p]:], sps[:, lo[p]:],
                                             ACTF.Exp, scale=SCALE)
                        if p >= 4 * tcx:
                            i = p - 4 * tcx
                            nc.vector.tensor_tensor(
                                pt[:, 128 * i:128 * (i + 1)],
                                pt[:, 128 * i:128 * (i + 1)],
                                tri_sb[:], AL.mult)
                        if p == 0:
                            nc.vector.tensor_copy(sumA[:], pt[:])
                        else:
                            nc.vector.tensor_tensor(
                                sumA[:, lo[p]:], sumA[:, lo[p]:],
                                pt[:, lo[p]:], AL.add)
                        pts[p] = pt
                        if p > 1:
                            nc.tensor.matmul(
                                avp[:, lo[p - 2]:], v_tiles[p - 2][:],
                                pts[p - 2][:, lo[p - 2]:],
                                start=(p == 2), stop=False)
                    for pp in (n_pt - 2, n_pt - 1):
                        nc.tensor.matmul(
                            avp[:, lo[pp]:], v_tiles[pp][:],
                            pts[pp][:, lo[pp]:],
                            start=(pp == 0), stop=(pp == n_pt - 1))
                    tail_a(tcx, h, aos, avp, sumA)
                    # dependency-free filler keeps the PE busy while the
                    # ACT/DVE softmax pipeline of this head drains
                    n_fill = 2 if tcx + 1 == nchunk else 1
                    for _ in range(n_fill):
                        if len(fillers) > reserve:
                            fillers.pop(0)()
                in_head[0] = False
                for fi, f in enumerate(fillers):
                    f()
                    if fi % 2 == 1:
                        drain_tail()
                pending_oproj.append((tcx, aos))

            # last chunk's o_proj: drain the remaining softmax tails, then
            # emit the final groups (their ao inputs are ready by now)
            while pending_tails:
                drain_tail(force=True)
            for optcx, opaos in pending_oproj:
                for j in range(4):
                    for q in range(4):
                        emit_oproj_group(optcx, opaos, j, q)
    nc.finalize()
    return nc


_NC_CACHE = None


def _get_nc():
    global _NC_CACHE
    if _NC_CACHE is None:
        _NC_CACHE = build()
    return _NC_CACHE


_HALF_PERM = np.concatenate([np.arange(0, HD, 2), np.arange(1, HD, 2)])


def _etile(a):
    """[D, C] -> [128, (D//128)*C]: row p, block e holds original row
    128e+p, so each DMA packet is a long contiguous run."""
    d, c = a.shape
    return np.ascontiguousarray(
        a.reshape(d // 128, 128, c).transpose(1, 0, 2).reshape(128, -1))


def _xtile(a):
    """[D, T] -> [128, cols] laid out pair-major then e-major: block
    (P, e) holds rows 128e+p, cols [1024P:1024(P+1)] — long contiguous
    runs AND chunk-pair arrival granularity."""
    d, t = a.shape
    pw = min(1024, t)
    return np.ascontiguousarray(
        a.reshape(d // 128, 128, t // pw, pw).transpose(1, 2, 0, 3)
        .reshape(128, -1))


def _shard_inputs(x, wq, wk, wv, wo):
    perm_q = np.concatenate([128 * h + _HALF_PERM for h in range(HPC)])
    in_maps = []
    for c in range(N_CORES):
        b, k = c // 4, c % 4
        xT = _xtile(np.ascontiguousarray(x[b].T.astype(BF16NP)))
        wq_c = _etile(
            wq[:, QCOLS * k:QCOLS * (k + 1)][:, perm_q].astype(BF16NP))
        wk_c = _etile(
            wk[:, HD * k:HD * (k + 1)][:, _HALF_PERM].astype(BF16NP))
        wv_c = _etile(wv[:, HD * k:HD * (k + 1)].astype(BF16NP))
        wo_c = np.ascontiguousarray(
            wo[QCOLS * k:QCOLS * (k + 1), :].astype(BF16NP))
        in_maps.append({"xT": xT, "wq": wq_c, "wk": wk_c, "wv": wv_c,
                        "wo": wo_c})
    return in_maps


def kernel(x, wq, wk, wv, wo, _trace=False, _trace_kwargs=None):
    x, wq, wk, wv, wo = (np.asarray(a, dtype=np.float32)
                         for a in (x, wq, wk, wv, wo))
    nc = _get_nc()
    in_maps = _shard_inputs(x, wq, wk, wv, wo)
    kw = {}
    if _trace:
        kw = dict(trace=True, **(_trace_kwargs or {}))
    res = bass_utils.run_bass_kernel_spmd(
        nc, in_maps, list(range(N_CORES)), **kw)
    t = x.shape[1]
    out = np.empty((B, t, D), np.float32)
    for b in range(B):
        acc = res.results[4 * b]["out"].astype(np.float32)
        for k in range(1, 4):
            acc += res.results[4 * b + k]["out"].astype(np.float32)
        out[b] = acc
    kernel.last_result = res
    return out
